# revision 16
# baseline (speedup 1.0000x reference)
"""Trainium2 Bass kernel for a 3-iteration custom transformer encoder layer.

Sharding: 8 cores = 4 batch groups x 2 cores. Within a pair, attention
queries / FFN positions are split in half. Host rotates each core's
sequence view so its local queries are always rotated positions 0..511,
making all 8 cores run one identical program on different data (no
collectives, no core-id branching). K/V are computed for the full
sequence on both cores of a pair (replicated compute).

Layout convention on device: activations are kept feature-on-partition
("transposed", [d, s]) so every matmul contraction dim lands on
partitions. Matmuls run as float32r (full PE rate at N>=256).
"""

import os
import sys
from contextlib import ExitStack

os.environ.setdefault("MYCRO_LOCAL_CACHE", "1")

for _p in ("/opt/trn_rl_repo", "/root/.axon_site/_ro/trn_rl_repo"):
    if os.path.isdir(_p) and _p not in sys.path:
        sys.path.insert(0, _p)

import numpy as np

import concourse.bass as bass
import concourse.tile as tile
from concourse import bacc, mybir
from concourse.bass_utils import run_bass_kernel_spmd

dt = mybir.dt
AF = mybir.ActivationFunctionType
ALU = mybir.AluOpType

# Problem shapes (hardcoded per contract)
B, S, D, H, DK, DFF, ITER = 4, 1024, 1024, 16, 64, 4096, 3
EPS = 1e-5
NEG = -9e15
INV_SQRT_DK = 1.0 / 8.0
N_CORES = 8
SQ = 512          # local queries per core
P = 128           # partitions
NKT = S // P      # 8 key tiles / d tiles
NST = SQ // P     # 4 local seq tiles
NFM = DFF // P    # 32 ff tiles

F32R = dt.float32r
F32 = dt.float32


def _r(ap):
    """View an AP as float32r for matmul operands."""
    return ap.bitcast(F32R)


def build_program():
    nc = bacc.Bacc("TRN2", target_bir_lowering=False, debug=False)

    def din(name, shape, dtype=F32):
        return nc.dram_tensor(name, list(shape), dtype, kind="ExternalInput").ap()

    d = dict(
        src=din("src", (S, D)),              # rotated batch slice
        mb=din("mb", (P, NKT)),              # mask bias (0 / NEG)
        wq=din("wq", (8, P, 1024), F32R),          # per head-pair lhsT blocks
        wk=din("wk", (8, P, 1024), F32R),
        wv=din("wv", (4, P, 2048), F32R),          # per head-quad
        wo=din("wo", (8, P, 1024), F32R),          # per dout-tile col-block
        w1=din("w1", (NFM, P, 1024), F32R),        # per ff-tile
        w2=din("w2", (DFF, D), F32R),
        bq=din("bq", (P, 8)),
        bk=din("bk", (P, 8)),
        bvb=din("bvb", (P, H * DK)),
        bo=din("bo", (P, 8)),
        b1=din("b1", (P, NFM)),
        b2=din("b2", (P, 8)),
        g1=din("g1", (P, 8)),
        b1l=din("b1l", (P, 8)),
        g2=din("g2", (P, 8)),
        b2l=din("b2l", (P, 8)),
        ident=din("ident", (P, P)),
        vones=din("vones", (P, H), F32R),
        out=nc.dram_tensor("out", [SQ, D], F32, kind="ExternalOutput").ap(),
    )

    with tile.TileContext(nc) as tc:
        _build(nc, tc, d)
    nc.compile()
    return nc


def _ln_stats(nc, pool, x_tile, width):
    """Return (rstd[P,1], nmr[P,1]) for rows of x_tile[:, :width]."""
    scr = pool.tile([P, width], F32, name="ln_scr", tag="ln_scr")
    sums = pool.tile([P, 1], F32, name="ln_sum", tag="ln_sum")
    sumsq = pool.tile([P, 1], F32, name="ln_ssq", tag="ln_ssq")
    # ACT passes: copy + row-sum, square + row-sum
    nc.scalar.activation(scr[:], x_tile[:, 0:width], AF.Copy, accum_out=sums[:])
    scr2 = pool.tile([P, width], F32, name="ln_scr2", tag="ln_scr2")
    nc.scalar.activation(scr2[:], x_tile[:, 0:width], AF.Square,
                         accum_out=sumsq[:])
    mean = pool.tile([P, 1], F32, name="ln_mean", tag="ln_mean")
    nc.vector.tensor_scalar_mul(mean[:], sums[:], 1.0 / width)
    m2 = pool.tile([P, 1], F32, name="ln_m2", tag="ln_m2")
    nc.vector.tensor_mul(m2[:], mean[:], mean[:])
    var = pool.tile([P, 1], F32, name="ln_var", tag="ln_var")
    nc.vector.tensor_scalar(var[:], sumsq[:], 1.0 / width, None, ALU.mult)
    nc.vector.tensor_sub(var[:], var[:], m2[:])
    sd = pool.tile([P, 1], F32, name="ln_sd", tag="ln_sd")
    nc.vector.tensor_scalar_add(sd[:], var[:], EPS)
    nc.scalar.sqrt(sd[:], sd[:])
    rstd = pool.tile([P, 1], F32, name="ln_rstd", tag="ln_rstd")
    nc.vector.reciprocal(rstd[:], sd[:])
    nmr = pool.tile([P, 1], F32, name="ln_nmr", tag="ln_nmr")
    nc.vector.tensor_mul(nmr[:], mean[:], rstd[:])
    nc.vector.tensor_scalar_mul(nmr[:], nmr[:], -1.0)
    return rstd, nmr


def _build(nc, tc, d):
    PH = int(os.environ.get("KERNEL_PHASES", "5"))

    def dump_debug(tiles, es_list):
        """Early-out: write 4 [128, 1024]-ish tiles to out and stop."""
        for st in range(NST):
            tl = tiles[st]
            w = tl.shape[-1]
            nc.sync.dma_start(d["out"][st * P:(st + 1) * P, 0:w],
                              tl[:].bitcast(F32) if tl.dtype == F32R else tl[:])
        for es in es_list:
            es.close()

    es0 = ExitStack()
    # ---- outermost long-lived pools (survive into FFN era) ----
    cpool = es0.enter_context(tc.tile_pool(name="consts", bufs=1))
    outt_pool = es0.enter_context(tc.tile_pool(name="outtp", bufs=1))

    def const(name, shape, src_ap):
        tl = cpool.tile(list(shape), F32, name=name)
        nc.sync.dma_start(tl[:], src_ap[:] if src_ap.shape == list(shape)
                          else src_ap)
        return tl

    mb = const("mb", (P, NKT), d["mb"])
    bq = const("bq", (P, 8), d["bq"])
    bk = const("bk", (P, 8), d["bk"])
    bvb = const("bvb", (P, H * DK), d["bvb"])
    bo = const("bo", (P, 8), d["bo"])
    b1c = const("b1c", (P, NFM), d["b1"])
    b2c = const("b2c", (P, 8), d["b2"])
    g1 = const("g1", (P, 8), d["g1"])
    b1l = const("b1l", (P, 8), d["b1l"])
    g2 = const("g2", (P, 8), d["g2"])
    b2l = const("b2l", (P, 8), d["b2l"])
    ident = const("ident", (P, P), d["ident"])

    outT = [outt_pool.tile([P, SQ], F32R, name=f"outT{i}") for i in range(NKT)]

    # ================= attention era =================
    es1 = ExitStack()
    xt_pool = es1.enter_context(tc.tile_pool(name="xt", bufs=1))
    kt_pool = es1.enter_context(tc.tile_pool(name="ktp", bufs=1))
    vn_pool = es1.enter_context(tc.tile_pool(name="vnp", bufs=1))
    ctx_pool = es1.enter_context(tc.tile_pool(name="ctxp", bufs=1))

    xT = [xt_pool.tile([P, S], F32R, name=f"xT{i}") for i in range(NKT)]
    kT = [kt_pool.tile([P, S], F32R, name=f"kT{i}") for i in range(8)]
    v_nat = [vn_pool.tile([P, H, DK + 1], F32R, name=f"vn{i}") for i in range(NKT)]
    ctxT = [ctx_pool.tile([P, SQ], F32R, name=f"ctxT{i}") for i in range(8)]

    for i in range(NKT):
        nc.sync.dma_start(v_nat[i][:, :, DK:DK + 1],
                          d["vones"].rearrange("p (h o) -> p h o", o=1))

    # ---- Phase 1: LN1 + transpose to xT ----
    with tc.tile_pool(name="srcs", bufs=2) as srcs_pool, \
         tc.tile_pool(name="lnw", bufs=2) as ln_pool, \
         tc.tile_pool(name="xn", bufs=2) as xn_pool, \
         tc.tile_pool(name="psT", bufs=2, space="PSUM") as psT_pool:
        for st in range(NKT):
            stile = srcs_pool.tile([P, D], F32, name="stile", tag="stile")
            nc.sync.dma_start(stile[:], d["src"][st * P:(st + 1) * P, :])
            rstd, nmr = _ln_stats(nc, ln_pool, stile, D)
            xn = xn_pool.tile([P, D], F32, name="xn", tag="xn")
            nc.scalar.activation(xn[:], stile[:], AF.Identity,
                                 bias=nmr[:], scale=rstd[:])
            for dtt in range(NKT):
                pst = psT_pool.tile([P, P], F32, name="pst", tag="pst")
                nc.tensor.transpose(pst[:], xn[:, dtt * P:(dtt + 1) * P],
                                    ident[:])
                nc.scalar.activation(
                    xT[dtt][:, st * P:(st + 1) * P], pst[:], AF.Identity,
                    bias=b1l[:, dtt:dtt + 1], scale=g1[:, dtt:dtt + 1])

    if PH == 1:
        return dump_debug(xT, [es1, es0])

    # ---- Phase 2: K / V projections (full S) ----
    with tc.tile_pool(name="kvw", bufs=2) as kvw_pool, \
         tc.tile_pool(name="psKV", bufs=3, space="PSUM") as psKV_pool:
        for pr in range(8):
            wkc = kvw_pool.tile([P, 8, P], F32R, name="wkc", tag="wkc")
            nc.sync.dma_start(wkc[:], d["wk"][pr].rearrange("p (k m) -> p k m", k=8))
            for half in range(2):
                psK = psKV_pool.tile([P, 512], F32, name="psK", tag="psK")
                for kt in range(NKT):
                    nc.tensor.matmul(
                        psK[:], _r(wkc[:, kt, :]),
                        _r(xT[kt][:, half * 512:(half + 1) * 512]),
                        start=(kt == 0), stop=(kt == NKT - 1))
                nc.scalar.activation(
                    kT[pr][:, half * 512:(half + 1) * 512], psK[:],
                    AF.Identity, bias=bk[:, pr:pr + 1])
        for quad in range(4):
            wvq = kvw_pool.tile([P, 8, 256], F32R, name="wvq", tag="wvq")
            nc.sync.dma_start(wvq[:], d["wv"][quad].rearrange("p (k m) -> p k m", k=8))
            for st in range(NKT):
                psV = psKV_pool.tile([P, 256], F32, name="psV", tag="psV")
                for kt in range(NKT):
                    nc.tensor.matmul(
                        psV[:], _r(xT[kt][:, st * P:(st + 1) * P]),
                        _r(wvq[:, kt, :]),
                        start=(kt == 0), stop=(kt == NKT - 1))
                nc.vector.tensor_add(
                    v_nat[st][:, 4 * quad:4 * quad + 4, 0:DK],
                    psV[:].rearrange("p (h k) -> p h k", h=4),
                    bvb[:, quad * 256:(quad + 1) * 256].rearrange(
                        "p (h k) -> p h k", h=4))

    if PH == 2:
        return dump_debug(kT, [es1, es0])

    # ---- Phase 3: attention iterations ----
    with tc.tile_pool(name="qw", bufs=2) as qw_pool, \
         tc.tile_pool(name="wos", bufs=2) as wos_pool, \
         tc.tile_pool(name="qt", bufs=2) as qt_pool, \
         tc.tile_pool(name="pp", bufs=3) as p_pool, \
         tc.tile_pool(name="rbp", bufs=2) as rb_pool, \
         tc.tile_pool(name="psQ", bufs=2, space="PSUM") as psQ_pool, \
         tc.tile_pool(name="psS", bufs=2, space="PSUM") as psS_pool, \
         tc.tile_pool(name="psC", bufs=2, space="PSUM") as psC_pool, \
         tc.tile_pool(name="psO", bufs=2, space="PSUM") as psO_pool:
        for it in range(ITER):
            for pr in range(8):
                wqc = qw_pool.tile([P, 8, P], F32R, name="wqc", tag="wqc")
                nc.sync.dma_start(
                    wqc[:], d["wq"][pr].rearrange("p (k m) -> p k m", k=8))
                psQ = psQ_pool.tile([P, SQ], F32, name="psQ", tag="psQ")
                for kt in range(NKT):
                    rhs = xT[kt][:, 0:SQ] if it == 0 else outT[kt][:]
                    nc.tensor.matmul(psQ[:], _r(wqc[:, kt, :]), _r(rhs),
                                     start=(kt == 0), stop=(kt == NKT - 1))
                qT = qt_pool.tile([P, SQ], F32R, name="qT", tag="qT")
                nc.scalar.activation(qT[:], psQ[:], AF.Identity,
                                     bias=bq[:, pr:pr + 1])
                for sub in range(2):
                    h = 2 * pr + sub
                    lo, hi = sub * 64, sub * 64 + 64
                    psC = psC_pool.tile([DK + 1, SQ], F32, name="psC", tag="psC")
                    for kt in range(NKT):
                        psS = psS_pool.tile([P, SQ], F32, name="psS", tag="psS")
                        nc.tensor.matmul(
                            psS[:], _r(kT[pr][lo:hi, kt * P:(kt + 1) * P]),
                            _r(qT[lo:hi, :]), start=True, stop=True)
                        pe = p_pool.tile([P, SQ], F32R, name="pe", tag="pe")
                        nc.scalar.activation(pe[:], psS[:], AF.Exp,
                                             bias=mb[:, kt:kt + 1])
                        nc.tensor.matmul(psC[:], _r(v_nat[kt][:, h, :]),
                                         _r(pe[:]),
                                         start=(kt == 0), stop=(kt == NKT - 1))
                    recip = rb_pool.tile([1, SQ], F32, name="recip", tag="recip")
                    nc.vector.reciprocal(recip[:], psC[DK:DK + 1, :])
                    rb = rb_pool.tile([64, SQ], F32, name="rb", tag="rb")
                    nc.gpsimd.partition_broadcast(rb[:], recip[:])
                    nc.vector.tensor_mul(ctxT[pr][lo:hi, :], psC[0:DK, :], rb[:])
            for mt in range(NKT):
                woc = wos_pool.tile([P, 8, P], F32R, name="woc", tag="woc")
                nc.sync.dma_start(
                    woc[:], d["wo"][mt].rearrange("p (k m) -> p k m", k=8))
                psO = psO_pool.tile([P, SQ], F32, name="psO", tag="psO")
                for kt in range(NKT):
                    nc.tensor.matmul(
                        psO[:], _r(woc[:, kt, :]),
                        _r(ctxT[kt][:]), start=(kt == 0), stop=(kt == NKT - 1))
                nc.scalar.activation(outT[mt][:], psO[:], AF.Identity,
                                     bias=bo[:, mt:mt + 1])

    if PH == 3:
        return dump_debug(outT, [es1, es0])

    es1.close()  # free xT / kT / v_nat / ctxT

    # ================= FFN era =================
    es2 = ExitStack()
    src2_pool = es2.enter_context(tc.tile_pool(name="src2p", bufs=1))
    yt_pool = es2.enter_context(tc.tile_pool(name="ytp", bufs=1))
    src2 = [src2_pool.tile([P, D], F32, name=f"src2_{i}") for i in range(NST)]
    yT = [yt_pool.tile([P, SQ], F32R, name=f"yT{i}") for i in range(NKT)]

    # ---- Phase 4: residual + LN2 + yT ----
    with tc.tile_pool(name="srcr", bufs=2) as srcr_pool, \
         tc.tile_pool(name="lnw2", bufs=2) as ln2_pool, \
         tc.tile_pool(name="x2n", bufs=2) as x2n_pool, \
         tc.tile_pool(name="psT2", bufs=2, space="PSUM") as psT2_pool:
        for st in range(NST):
            sres = srcr_pool.tile([P, D], F32, name="sres", tag="sres")
            nc.sync.dma_start(sres[:], d["src"][st * P:(st + 1) * P, :])
            for dtt in range(NKT):
                pst = psT2_pool.tile([P, P], F32, name="pst2", tag="pst2")
                nc.tensor.transpose(pst[:], outT[dtt][:, st * P:(st + 1) * P].bitcast(F32),
                                    ident[:])
                nc.vector.tensor_add(
                    src2[st][:, dtt * P:(dtt + 1) * P], pst[:],
                    sres[:, dtt * P:(dtt + 1) * P])
            rstd2, nmr2 = _ln_stats(nc, ln2_pool, src2[st], D)
            x2n = x2n_pool.tile([P, D], F32, name="x2n", tag="x2n")
            nc.scalar.activation(x2n[:], src2[st][:], AF.Identity,
                                 bias=nmr2[:], scale=rstd2[:])
            for dtt in range(NKT):
                pst = psT2_pool.tile([P, P], F32, name="pst2", tag="pst2")
                nc.tensor.transpose(pst[:], x2n[:, dtt * P:(dtt + 1) * P],
                                    ident[:])
                nc.scalar.activation(
                    yT[dtt][:, st * P:(st + 1) * P], pst[:], AF.Identity,
                    bias=b2l[:, dtt:dtt + 1], scale=g2[:, dtt:dtt + 1])

    if PH == 4:
        return dump_debug(src2, [es2, es0])

    # ---- Phase 5: FFN ----
    with tc.tile_pool(name="ht", bufs=1) as ht_pool:
        hT = [ht_pool.tile([P, SQ], F32R, name=f"hT{i}") for i in range(NFM)]
        with tc.tile_pool(name="w1s", bufs=3) as w1_pool, \
             tc.tile_pool(name="psH", bufs=2, space="PSUM") as psH_pool:
            for fm in range(NFM):
                w1c = w1_pool.tile([P, 8, P], F32R, name="w1c", tag="w1c")
                nc.sync.dma_start(
                    w1c[:], d["w1"][fm].rearrange("p (k m) -> p k m", k=8))
                psH = psH_pool.tile([P, SQ], F32, name="psH", tag="psH")
                for kt in range(NKT):
                    nc.tensor.matmul(psH[:], _r(w1c[:, kt, :]), _r(yT[kt][:]),
                                     start=(kt == 0), stop=(kt == NKT - 1))
                nc.scalar.activation(hT[fm][:], psH[:], AF.Relu,
                                     bias=b1c[:, fm:fm + 1])
        y2T = [ht_pool.tile([P, SQ], F32, name=f"y2T{i}") for i in range(NKT)]
        with tc.tile_pool(name="w2s", bufs=3) as w2_pool, \
             tc.tile_pool(name="psY", bufs=1, space="PSUM") as psY_pool:
            psY = psY_pool.tile([P, NKT, SQ], F32, name="psY")
            for kt in range(NFM):
                w2r = w2_pool.tile([P, D], F32R, name="w2r", tag="w2r")
                nc.sync.dma_start(w2r[:], d["w2"][kt * P:(kt + 1) * P, :])
                for mt in range(NKT):
                    nc.tensor.matmul(
                        psY[:, mt, :], _r(w2r[:, mt * P:(mt + 1) * P]),
                        _r(hT[kt][:]), start=(kt == 0), stop=(kt == NFM - 1))
            for mt in range(NKT):
                nc.scalar.activation(y2T[mt][:], psY[:, mt, :], AF.Identity,
                                     bias=b2c[:, mt:mt + 1])

        # final: out = src2 + y2 (transpose back to natural, fused add)
        with tc.tile_pool(name="fin", bufs=2) as fin_pool, \
             tc.tile_pool(name="psT3", bufs=2, space="PSUM") as psT3_pool:
            for st in range(NST):
                fin = fin_pool.tile([P, D], F32, name="fin", tag="fin")
                for dtt in range(NKT):
                    pst = psT3_pool.tile([P, P], F32, name="pst3", tag="pst3")
                    nc.tensor.transpose(
                        pst[:], y2T[dtt][:, st * P:(st + 1) * P], ident[:])
                    nc.vector.tensor_add(
                        fin[:, dtt * P:(dtt + 1) * P], pst[:],
                        src2[st][:, dtt * P:(dtt + 1) * P])
                nc.sync.dma_start(d["out"][st * P:(st + 1) * P, :], fin[:])

    es2.close()
    es0.close()


# ---------------------------------------------------------------------------
# Host side
# ---------------------------------------------------------------------------

def make_core_inputs(src, mask, Wq, bq, Wk, bk, Wv, bv, scale, Wo, bo,
                     ln1_g, ln1_b, ln2_g, ln2_b, W1, b1, W2, b2):
    """Build the 8 per-core input maps (numpy, fp32)."""
    f = np.float32
    src = np.asarray(src, f)
    mask = np.asarray(mask)

    def pack_col_blocks(w, nblk, blk):
        # w: [D, M_total] -> [nblk, P, (D//P)*blk] contiguous per col-block
        return np.ascontiguousarray(
            w.reshape(D // P, P, nblk, blk).transpose(2, 1, 0, 3)
            .reshape(nblk, P, (D // P) * blk))

    wq_flat = (np.asarray(Wq, f) * INV_SQRT_DK).transpose(1, 0, 2).reshape(D, H * DK)
    wk_flat = np.asarray(Wk, f).transpose(1, 0, 2).reshape(D, H * DK)
    wv_s = np.asarray(Wv, f) * np.asarray(scale, f)[:, None, None]
    wv_flat = wv_s.transpose(1, 0, 2).reshape(D, H * DK)
    wq_p = pack_col_blocks(wq_flat, 8, P)
    wk_p = pack_col_blocks(wk_flat, 8, P)
    wv_p = pack_col_blocks(wv_flat, 4, 256)
    wo_p = pack_col_blocks(np.asarray(Wo, f), 8, P)
    w1_p = pack_col_blocks(np.asarray(W1, f), NFM, P)
    w2_n = np.ascontiguousarray(np.asarray(W2, f))

    def cols(vec):  # [1024] -> [128, 8] tile-column layout
        return np.ascontiguousarray(np.asarray(vec, f).reshape(-1, P).T)

    bq_c = cols((np.asarray(bq, f) * INV_SQRT_DK).reshape(H * DK))
    bk_c = cols(np.asarray(bk, f).reshape(H * DK))
    bv_s = (np.asarray(bv, f) * np.asarray(scale, f)[:, None]).reshape(H * DK)
    bvb = np.ascontiguousarray(np.broadcast_to(bv_s[None, :], (P, H * DK)))
    shared = dict(wq=wq_p, wk=wk_p, wv=wv_p, wo=wo_p, w1=w1_p, w2=w2_n,
                  bq=bq_c, bk=bk_c, bvb=bvb, bo=cols(np.asarray(bo, f)),
                  b1=cols(np.asarray(b1, f)), b2=cols(np.asarray(b2, f)),
                  g1=cols(np.asarray(ln1_g, f)), b1l=cols(np.asarray(ln1_b, f)),
                  g2=cols(np.asarray(ln2_g, f)), b2l=cols(np.asarray(ln2_b, f)),
                  ident=np.eye(P, dtype=f), vones=np.ones((P, H), dtype=f))

    in_maps = []
    for c in range(N_CORES):
        b, p = c // 2, c % 2
        roll = p * SQ
        src_rot = np.ascontiguousarray(np.roll(src[b], -roll, axis=0))
        mask_rot = np.roll(np.asarray(mask)[b], -roll)
        mbias = np.where(mask_rot == 0, np.float32(NEG), np.float32(0.0))
        mb_t = np.ascontiguousarray(mbias.reshape(NKT, P).T.astype(f))
        m = dict(shared)
        m["src"] = src_rot
        m["mb"] = mb_t
        in_maps.append(m)
    return in_maps


_NC = None
_last_results = None


def kernel(**inputs):
    global _NC, _last_results
    if _NC is None:
        _NC = build_program()
    in_maps = make_core_inputs(**inputs)
    trace = bool(int(os.environ.get("KERNEL_TRACE", "0")))
    res = run_bass_kernel_spmd(_NC, in_maps, core_ids=list(range(N_CORES)),
                               trace=trace)
    _last_results = res
    out = np.empty((B, S, D), np.float32)
    for c in range(N_CORES):
        b, p = c // 2, c % 2
        out[b, p * SQ:(p + 1) * SQ, :] = res.results[c]["out"]
    return out


if __name__ == "__main__":
    nc = build_program()
    print("build OK")
